# revision 1
# baseline (speedup 1.0000x reference)
"""Trainium2 Bass kernel for nn_DirectEncodingModel (gnn_message_passing).

Model (reference):
    h = x                                  # [B, 256]
    for l in 0..2:
        gathered = h[:, idx[l]]            # [B, 4, 128]
        z = einsum('bgk,gku->bgu', gathered, W[l]) + b[l]
        h = tanh(z).reshape(B, 256)
    out = h @ W_out + b_out                # [B, 10]

Key transforms (host-side, exact):
  * levels 1-2: the gather folds into a dense weight matrix per level,
        Weff[l][d, g*64+u] = sum_{k: idx[l,g,k]==d} W[l,g,k,u]
    so each level is h = tanh(h @ Weff[l] + b[l]) — a dense
    [B,256]@[256,256] matmul (the gather acts on on-chip activations, so
    folding it into the contraction is the only gather-free form).
  * level 0: the gather acts on x, so the host pre-gathers x per group
    (xg[g] = x[:, idx[0,g]]) and the device runs one K=128, M=64 matmul
    per group with the raw W[0,g] weights; the two M=64 halves of a pair
    occupy distinct PE column groups (tile_position via base partitions)
    and stream concurrently — half the PE cycles of the dense form.

Device layout: activations transposed — [feature(partition), batch(free)].
Host pre-transposes x (and casts to fp16); device does fp16 matmuls
(1 cycle/row on the PE, same as bf16, 3 more mantissa bits) with fp32
PSUM accumulation, tanh on the scalar engine, and writes out^T [10, BS];
host transposes back and adds b_out (exact: b_out is a constant
broadcast, added in fp32 on host).

The per-chunk schedule is explicitly software-pipelined (skewed emission:
out(i-3) | L2(i-2) | L1(i-1) | L0(i) per tick) so each engine's in-order
stream never blocks on same-tick producers. PSUM budget (8 banks):
3 z-slots of 2 banks (pipeline depth 3) + 2 out-slots of 1 bank.
Steady state: ScalarE (tanh, 12.6M elems/core at 1 elem/lane/cycle =
82 us floor) and TensorE (~83 us with col-tiled L0) are both near
saturation; measured ~84-99 us/rep steady-state (paired-slope), 118 us
single-shot in the cost model incl. pipeline fill + teardown (the model
does not credit the L0 column-tiling concurrency, so true single-shot
is ~105-110 us). x loads are one DMA per block ([128, G, blk]
partition-major), weights load interleaved with the first x blocks in
FIFO first-use order.

Sharding: pure data parallelism over the batch axis across 8 cores;
weights replicated.
"""

import numpy as np

import concourse.mybir as mybir
import concourse.bacc as bacc
import concourse.tile as tile
from concourse.bass_utils import run_bass_kernel_spmd

F16 = mybir.dt.float16
F32 = mybir.dt.float32

N_CORES = 8
B, D, L, G, K, U, OUT = 131072, 256, 3, 4, 128, 64, 10
GU = G * U  # 256
BS = B // N_CORES  # 16384 per core

CHUNK = 512           # batch columns per level-computation (one PSUM slot)
NCHUNK = BS // CHUNK  # 32
XBLK = 1024           # batch columns per x DMA
OBLK = 1024           # batch columns per output DMA

# test-harness hooks (harness never touches these; defaults are production)
TRACE = False
LAST_RESULTS = None

_PROG_CACHE = {}


def _build_program(use_bias: bool, reps: int = 1):
    nc = bacc.Bacc("TRN2", debug=False, target_bir_lowering=False,
                   num_devices=N_CORES)

    # level 0 uses the host-pre-gathered x (one K=128 matmul per group,
    # M=64, pairs run concurrently via PE column tiling); levels 1-2 use
    # the dense folded weights
    xg_d = nc.dram_tensor("xg", [128, G, BS], F16, kind="ExternalInput")
    w0_d = nc.dram_tensor("w0", [128, G, U], F16, kind="ExternalInput")
    weff_d = nc.dram_tensor("weff", [128, 2 * (L - 1), GU], F16,
                            kind="ExternalInput")
    wout_d = nc.dram_tensor("wout", [128, 2, OUT], F16, kind="ExternalInput")
    if use_bias:
        bias_d = nc.dram_tensor("bias", [128, 2 * L], F32, kind="ExternalInput")
    outt_d = nc.dram_tensor("outt", [OUT, BS], F32, kind="ExternalOutput")

    Tanh = mybir.ActivationFunctionType.Tanh

    with tile.TileContext(nc) as tc:
        with tc.tile_pool(name="const", bufs=1) as cpool, \
             tc.tile_pool(name="xp", bufs=5) as xpool, \
             tc.tile_pool(name="hp", bufs=4) as hpool, \
             tc.tile_pool(name="obp", bufs=2) as obpool, \
             tc.tile_pool(name="zp", bufs=3, space="PSUM") as zpool, \
             tc.tile_pool(name="op", bufs=2, space="PSUM") as opool:

            # level-0 weights only; the sync engine's HWDGE ring is FIFO, so
            # the big weff load is deferred until after the first x blocks
            # (weff isn't consumed until tick 1)
            # w0 is small and feeds the very first matmuls: sync ring, first
            # (SWDGE/scalar-ring/interleaved-halves variants all modeled
            # worse — see session notes)
            w0_t = cpool.tile([128, G, U], F16)
            nc.sync.dma_start(w0_t[:, :, :], w0_d[:, :, :])
            weff_t = cpool.tile([128, 2 * (L - 1), GU], F16)
            wout_t = cpool.tile([128, 2, OUT], F16)
            if use_bias:
                bias_t = cpool.tile([128, 2 * L], F32)

            # trigger the ACT tanh table-set load immediately so it overlaps
            # the first x DMA instead of stalling the first real activation
            warm_in = cpool.tile([128, 1], F32)
            warm_out = cpool.tile([128, 1], F16)
            nc.gpsimd.memset(warm_in[:, :], 0.0)
            nc.scalar.activation(warm_out[:, :], warm_in[:, :], Tanh)

            # x DMA blocks: first two at chunk granularity so the pipeline
            # fills fast, the rest at XBLK
            xblocks = [(0, CHUNK), (CHUNK, CHUNK)]
            off = 2 * CHUNK
            while off < BS:
                sz = min(XBLK, BS - off)
                xblocks.append((off, sz))
                off += sz
            chunk_block = {}
            for bi, (s, sz) in enumerate(xblocks):
                for c in range(s // CHUNK, (s + sz) // CHUNK):
                    chunk_block[c] = bi

            for _rep in range(reps):
                # software-pipelined over chunks: at tick i we emit
                #   out(i-3) | L2(i-2) | L1(i-1) | L0(i)
                # so every instruction in a tick is dep-ready at tick start
                # (its producers ran in earlier ticks) — each engine's
                # in-order stream never head-of-line blocks.
                xts = {}
                hs = [{} for _ in range(L)]  # hs[l][c] = tile holding h_{l+1}(c)
                obs = {}

                def load_x(c):
                    bi = chunk_block[c]
                    if bi in xts:
                        return
                    s, sz = xblocks[bi]
                    t = xpool.tile([128, G, sz], F16, tag="x",
                                   name=f"xr{_rep}b{bi}",
                                   padded_shape=[128, G, XBLK])
                    if bi == 0 and _rep == 0:
                        # split the very first load by group pair so the
                        # first L0 matmul pair starts after half the data
                        nc.sync.dma_start(t[:, 0:2, :],
                                          xg_d[:, 0:2, s:s + sz])
                        nc.sync.dma_start(t[:, 2:4, :],
                                          xg_d[:, 2:4, s:s + sz])
                    else:
                        nc.sync.dma_start(t[:, :, :], xg_d[:, :, s:s + sz])
                    xts[bi] = t

                def level(c, l):
                    z = zpool.tile([128, 2, CHUNK], F32, tag="z",
                                   name=f"zr{_rep}c{c}l{l}")
                    if l == 0:
                        # gathered form: one K=128 matmul per group; the two
                        # M=64 halves of each pair land in distinct PE column
                        # groups (tile_position from base partitions) and run
                        # concurrently
                        bi = chunk_block[c]
                        s, sz = xblocks[bi]
                        xoff = c * CHUNK - s
                        for pair in range(2):
                            for j in range(2):
                                g = 2 * pair + j
                                nc.tensor.matmul(
                                    z[64 * j:64 * (j + 1), pair, :],
                                    w0_t[:, g, :],
                                    xts[bi][:, g, xoff:xoff + CHUNK],
                                    start=True, stop=True)
                    else:
                        for mt in range(2):
                            for kt in range(2):
                                rhs = hs[l - 1][c][:, kt, :]
                                nc.tensor.matmul(
                                    z[:, mt, :],
                                    weff_t[:, (l - 1) * 2 + kt,
                                           mt * 128:(mt + 1) * 128],
                                    rhs,
                                    start=(kt == 0), stop=(kt == 1))
                    hcur = hpool.tile([128, 2, CHUNK], F16, tag=f"h{l}",
                                      name=f"hr{_rep}c{c}l{l}")
                    if use_bias:
                        for mt in range(2):
                            nc.scalar.activation(
                                hcur[:, mt, :], z[:, mt, :], Tanh,
                                bias=bias_t[:, l * 2 + mt:l * 2 + mt + 1])
                    else:
                        nc.scalar.activation(hcur[:, :, :], z[:, :, :], Tanh)
                    hs[l][c] = hcur
                    if l > 0:
                        del hs[l - 1][c]

                def out_layer(c):
                    po = opool.tile([OUT, CHUNK], F32, tag="po",
                                    name=f"por{_rep}c{c}")
                    for kt in range(2):
                        nc.tensor.matmul(po[:, :], wout_t[:, kt, :],
                                         hs[L - 1][c][:, kt, :],
                                         start=(kt == 0), stop=(kt == 1))
                    del hs[L - 1][c]
                    oblk = c // (OBLK // CHUNK)
                    if c % (OBLK // CHUNK) == 0:
                        obs[oblk] = obpool.tile([OUT, OBLK], F32, tag="ob",
                                                name=f"obr{_rep}b{oblk}")
                    ooff = (c % (OBLK // CHUNK)) * CHUNK
                    nc.vector.tensor_copy(obs[oblk][:, ooff:ooff + CHUNK],
                                          po[:, :])
                    if c % (OBLK // CHUNK) == (OBLK // CHUNK) - 1:
                        # out-stores ride the idle GpSimd SWDGE path so the
                        # sync HWDGE FIFO carries only latency-sensitive
                        # x loads; the final store stays on HWDGE (lower
                        # completion latency — the teardown waits on it)
                        eng = nc.sync if c == NCHUNK - 1 else nc.gpsimd
                        eng.dma_start(
                            outt_d[:, oblk * OBLK:(oblk + 1) * OBLK],
                            obs[oblk][:, :])
                        del obs[oblk]

                load_x(0)  # prologue prefetch
                if _rep == 0:
                    # weff l=1 half before x block 1 (first used at tick 1),
                    # the rest behind it — FIFO order of first use
                    nc.sync.dma_start(weff_t[:, 0:2, :], weff_d[:, 0:2, :])
                load_x(1)
                if _rep == 0:
                    nc.sync.dma_start(weff_t[:, 2:4, :], weff_d[:, 2:4, :])
                    nc.sync.dma_start(wout_t[:, :, :], wout_d[:, :, :])
                    if use_bias:
                        nc.sync.dma_start(bias_t[:, :], bias_d[:, :])
                for i in range(NCHUNK + L):
                    if i - L >= 0:
                        out_layer(i - L)
                    for l in range(L - 1, -1, -1):
                        c = i - l
                        if 0 <= c < NCHUNK:
                            level(c, l)
                    for ahead in (1, 2, 3):
                        if i + ahead < NCHUNK:
                            load_x(i + ahead)

    nc.compile()
    return nc


def _prepare_in_maps(x, idx, W, b, W_out):
    """Host-side prep: weight folding, layouts, shard + transpose + cast."""
    # fold the gather into dense per-level weights for levels 1..L-1
    # (exact, fp32); level 0 keeps raw per-group weights and uses
    # host-pre-gathered x instead
    Weff = np.zeros((L - 1, D, GU), np.float32)
    for l in range(1, L):
        for g in range(G):
            np.add.at(Weff[l - 1, :, g * U:(g + 1) * U], idx[l, g], W[l, g])

    # device weight layouts (K-tile on partitions)
    weff_dev = np.ascontiguousarray(
        Weff.reshape(L - 1, 2, 128, GU).transpose(2, 0, 1, 3)
        .reshape(128, 2 * (L - 1), GU)).astype(np.float16)
    w0_dev = np.ascontiguousarray(
        W[0].transpose(1, 0, 2)).astype(np.float16)       # [128, G, U]
    wout_dev = np.ascontiguousarray(
        W_out.reshape(2, 128, OUT).transpose(1, 0, 2)).astype(
        np.float16)
    idx0 = idx[0].reshape(-1)                             # [G*K]

    use_bias = bool(np.any(b != 0.0))
    bias_dev = np.ascontiguousarray(
        b.reshape(L, 2, 128).transpose(2, 0, 1).reshape(128, 2 * L)) \
        if use_bias else None

    in_maps = []
    for c in range(N_CORES):
        xs = x[c * BS:(c + 1) * BS]                       # [BS, 256]
        xt = xs.T.astype(np.float16)                      # [256, BS] contig
        # gathered, partition-major [128, G, BS]
        xg = xt[idx0].reshape(G, 128, BS).transpose(1, 0, 2)
        m = {"xg": np.ascontiguousarray(xg),
             "w0": w0_dev, "weff": weff_dev, "wout": wout_dev}
        if use_bias:
            m["bias"] = bias_dev
        in_maps.append(m)
    return in_maps, use_bias


def kernel(x, idx, W, b, W_out, b_out):
    global LAST_RESULTS
    x = np.asarray(x, dtype=np.float32)
    idx = np.asarray(idx, dtype=np.int32)
    W = np.asarray(W, dtype=np.float32)
    b = np.asarray(b, dtype=np.float32)
    W_out = np.asarray(W_out, dtype=np.float32)
    b_out = np.asarray(b_out, dtype=np.float32)

    in_maps, use_bias = _prepare_in_maps(x, idx, W, b, W_out)

    nc = _PROG_CACHE.get(use_bias)
    if nc is None:
        nc = _PROG_CACHE[use_bias] = _build_program(use_bias)

    res = run_bass_kernel_spmd(nc, in_maps, list(range(N_CORES)),
                               trace=TRACE)
    LAST_RESULTS = res

    out = np.empty((B, OUT), np.float32)
    for c in range(N_CORES):
        out[c * BS:(c + 1) * BS] = res.results[c]["outt"].T
    if np.any(b_out != 0.0):
        out += b_out[None, :]
    return out

